# revision 7
# baseline (speedup 1.0000x reference)
"""Trainium2 Bass kernel for CGRCNet-style cold-item scoring.

Computes, for U=2048 users and C=1024 cold items:
    x        = item_content @ Wi.T + bi          (only the cold rows are needed)
    xc       = x[cold_ids]                        (C, D)
    hu       = h_u_bar @ W1h.T                    (U, H)
    hx       = xc @ W1x.T                         (C, H)
    logits   = einsum('uch,h->uc', relu(hu[:,None,:] + hx[None,:,:] + b1), W2[0]) + b2

Sharding: U across 8 cores (256 users/core); everything else replicated.
The cold-row gather (zero FLOPs) happens on the host as part of input
distribution; all matrix math runs on device.

Device-side plan (per core), layouts transposed so H lives on partitions:
  stage 1: xcT  (D=64p,  C=1024f) = WiT.T @ xcgT   (K=300 in 3 chunks) + bi
  stage 2: hxbT (H=128p, C=1024f) = W1xT.T @ xcT + b1          [f16]
  stage 3: huT  (H=128p, U=256f)  = W1hT.T @ huT_in
  main loop over cohorts of 128 users (32 waves x 4 PE col groups):
    R_u = relu(hxbT + huT[:,u])   produced on DVE / ACT / Pool per a
                                  tunable split (all three engines run
                                  elementwise in parallel; DVE gets the
                                  4x f16 mode, so it takes the most)
    logits accumulation on PE: per user one (128,32)->psum matvec per
      C-half; 4 col groups run CONCURRENTLY on HW (independent XBUS
      streams), so PE time ~ 2x512 cols per wave of 4 users.
    b2 is folded into the psum init wave (rank-1 matmul of b2 x ones).
    eviction: PSUM -> DRAM direct DMA (no vector-engine eviction work).
"""

import os
import numpy as np

# ---------------- problem constants (hardcoded per contract) ----------------
U, D = 2048, 64
I_ITEMS, CD = 50000, 300
C = 1024
H = 128
N_CORES = 8
UL = U // N_CORES            # 256 users per core
COHORT = 128                 # users per cohort (4 col groups x 32 waves)
WAVES = 32
GROUPS = 4
HALF = 512                   # free-dim half (PSUM bank = 512 fp32)

# ---------------- tunables ----------------
# producers per 32 users: "dve,act,pool" counts (must sum to 32)
SPLIT = os.environ.get("KRN_SPLIT", "26,6,0")
# eviction staging engine: "dve" | "act" | "pool"
EV_MODE = os.environ.get("KRN_EV", "act")
# engine for stage evictions: "act" or "dve"
STG_EV = os.environ.get("KRN_STG_EV", "act")
# R tile pool depth
RBUFS = int(os.environ.get("KRN_RBUFS", "10"))
# psum packing: dense (4 groups share a bank) or sparse (1 group per bank)
DENSE = os.environ.get("KRN_DENSE", "1") == "1"

_CACHE = {}


def _mk_pattern(split):
    """Evenly interleaved engine pattern of length 32 from counts."""
    nd, na, np_ = (int(x) for x in split.split(","))
    assert nd + na + np_ == 32
    slots = []
    for eng, n in (("d", nd), ("a", na), ("p", np_)):
        for i in range(n):
            slots.append(((i + 0.5) / n if n else 1e9, eng))
    slots.sort()
    return [e for _, e in slots]


def build_bass(reps=1, split=None, ev_mode=None, stg_ev=None, rbufs=None,
               dense=None):
    split = split if split is not None else SPLIT
    ev_mode = ev_mode if ev_mode is not None else EV_MODE
    stg_ev = stg_ev if stg_ev is not None else STG_EV
    rbufs = rbufs if rbufs is not None else RBUFS
    dense = dense if dense is not None else DENSE
    key = (split, ev_mode, stg_ev, rbufs, dense, reps)
    if key in _CACHE:
        return _CACHE[key]

    import concourse.bacc as bacc
    import concourse.mybir as mybir
    from concourse import tile

    F32 = mybir.dt.float32
    F16 = mybir.dt.float16
    ADD = mybir.AluOpType.add
    MAX = mybir.AluOpType.max
    RELU = mybir.ActivationFunctionType.Relu
    IDENT = mybir.ActivationFunctionType.Identity

    pattern = _mk_pattern(split)

    nc = bacc.Bacc("TRN2", target_bir_lowering=False, debug=False,
                   num_devices=N_CORES)

    # ---- DRAM tensors (names are the in_map keys) ----
    xcgT_d = nc.dram_tensor("xcgT", [CD, C], F16, kind="ExternalInput").ap()
    wiT_d = nc.dram_tensor("wiT", [CD, D], F16, kind="ExternalInput").ap()
    bicol_d = nc.dram_tensor("bicol", [D, 1], F32, kind="ExternalInput").ap()
    w1xT_d = nc.dram_tensor("w1xT", [D, H], F32, kind="ExternalInput").ap()
    w1hT_d = nc.dram_tensor("w1hT", [D, H], F32, kind="ExternalInput").ap()
    b1col_d = nc.dram_tensor("b1col", [H, 1], F32, kind="ExternalInput").ap()
    huT_d = nc.dram_tensor("huT", [D, UL], F32, kind="ExternalInput").ap()
    w2big_d = nc.dram_tensor("w2big", [H, WAVES * 32], F16,
                             kind="ExternalInput").ap()
    b2row_d = nc.dram_tensor("b2row", [1, 128], F16, kind="ExternalInput").ap()
    ones1_d = nc.dram_tensor("ones1", [1, HALF], F16, kind="ExternalInput").ap()
    logits_d = nc.dram_tensor("logits", [UL, C], F32, kind="ExternalOutput").ap()

    KCH = [(0, 128), (128, 128), (256, CD - 256)]  # K chunks of CD=300

    def stage_evict(dst, src, bias):
        if stg_ev == "act":
            if bias is None:
                nc.scalar.copy(dst, src)
            else:
                nc.scalar.activation(dst, src, IDENT, bias=bias, scale=1.0)
        else:
            if bias is None:
                nc.vector.tensor_copy(dst, src)
            else:
                nc.vector.tensor_scalar(dst, src, bias, None, ADD)

    with tile.TileContext(nc) as tc:
        with (
            tc.tile_pool(name="const", bufs=1) as constp,
            tc.tile_pool(name="work", bufs=1) as workp,
            tc.tile_pool(name="rpool", bufs=rbufs) as rpool,
            tc.tile_pool(name="evpool", bufs=4) as evpool,
        ):
            for rep in range(reps):
                # ---- load replicated operands ----
                xcgT_sb = []
                wiT_sb = []
                for i, (k0, kn) in enumerate(KCH):
                    t = constp.tile([kn, C], F16, name=f"rep{rep}_xcgT{i}",
                                    tag=f"xcg{i}")
                    nc.sync.dma_start(t[:, :], xcgT_d[k0:k0 + kn, :])
                    xcgT_sb.append(t)
                    w = constp.tile([kn, D], F16, name=f"rep{rep}_wiT{i}",
                                    tag=f"wiT{i}")
                    nc.sync.dma_start(w[:, :], wiT_d[k0:k0 + kn, :])
                    wiT_sb.append(w)
                w1xT_sb = constp.tile([D, H], F32, name=f"rep{rep}_w1xT",
                                      tag="w1xT")
                nc.sync.dma_start(w1xT_sb[:, :], w1xT_d[:, :])
                w1hT_sb = constp.tile([D, H], F32, name=f"rep{rep}_w1hT",
                                      tag="w1hT")
                nc.sync.dma_start(w1hT_sb[:, :], w1hT_d[:, :])
                huTin_sb = constp.tile([D, UL], F32, name=f"rep{rep}_huTin",
                                       tag="huTin")
                nc.sync.dma_start(huTin_sb[:, :], huT_d[:, :])
                b1col_sb = constp.tile([H, 1], F32, name=f"rep{rep}_b1col",
                                       tag="b1col")
                nc.sync.dma_start(b1col_sb[:, :], b1col_d[:, :])
                bicol_sb = constp.tile([D, 1], F32, name=f"rep{rep}_bicol",
                                       tag="bicol")
                nc.sync.dma_start(bicol_sb[:, :], bicol_d[:, :])
                w2big_sb = constp.tile([H, WAVES * 32], F16,
                                       name=f"rep{rep}_w2big", tag="w2big")
                nc.sync.dma_start(w2big_sb[:, :], w2big_d[:, :])
                b2row_sb = constp.tile([1, 128], F16, name=f"rep{rep}_b2row",
                                       tag="b2row")
                nc.sync.dma_start(b2row_sb[:, :], b2row_d[:, :])
                ones1_sb = constp.tile([1, HALF], F16, name=f"rep{rep}_ones1",
                                       tag="ones1")
                nc.sync.dma_start(ones1_sb[:, :], ones1_d[:, :])

                # ---- stages ----
                xcT_sb = workp.tile([D, C], F32, name=f"rep{rep}_xcT",
                                    tag="xcT")
                hxbT_sb = workp.tile([H, C], F16, name=f"rep{rep}_hxbT",
                                     tag="hxbT")
                huT_sb = workp.tile([H, UL], F32, name=f"rep{rep}_huT",
                                    tag="huT")
                with tc.tile_pool(name=f"rep{rep}_pstg", bufs=2,
                                  space="PSUM") as pstg:
                    for n in range(2):
                        ps1 = pstg.tile([H, HALF], F32,
                                        name=f"rep{rep}_ps_s1_{n}", tag="pstg")
                        for k, (k0, kn) in enumerate(KCH):
                            nc.tensor.matmul(
                                ps1[0:D, :], wiT_sb[k][:, :],
                                xcgT_sb[k][:, n * HALF:(n + 1) * HALF],
                                start=(k == 0), stop=(k == len(KCH) - 1))
                        stage_evict(xcT_sb[:, n * HALF:(n + 1) * HALF],
                                    ps1[0:D, :], bicol_sb[:, 0:1])
                    for n in range(2):
                        ps2 = pstg.tile([H, HALF], F32,
                                        name=f"rep{rep}_ps_s2_{n}", tag="pstg")
                        nc.tensor.matmul(
                            ps2[:, :], w1xT_sb[:, :],
                            xcT_sb[:, n * HALF:(n + 1) * HALF],
                            start=True, stop=True)
                        stage_evict(hxbT_sb[:, n * HALF:(n + 1) * HALF],
                                    ps2[:, :], b1col_sb[:, 0:1])
                    ps3 = pstg.tile([H, HALF], F32, name=f"rep{rep}_ps_s3",
                                    tag="pstg")
                    nc.tensor.matmul(ps3[:, 0:UL], w1hT_sb[:, :],
                                     huTin_sb[:, :], start=True, stop=True)
                    stage_evict(huT_sb[:, :], ps3[:, 0:UL], None)

                # ---- main loop ----
                n_cohorts = UL // COHORT

                def produce(u, idx, rt):
                    eng = pattern[idx % 32]
                    if eng == "d":
                        nc.vector.tensor_scalar(
                            rt[:, :], hxbT_sb[:, :], huT_sb[:, u:u + 1],
                            0.0, ADD, MAX)
                    elif eng == "a":
                        nc.scalar.activation(
                            rt[:, :], hxbT_sb[:, :], RELU,
                            bias=huT_sb[:, u:u + 1], scale=1.0)
                    else:
                        nc.gpsimd.tensor_scalar(
                            rt[:, :], hxbT_sb[:, :], huT_sb[:, u:u + 1],
                            0.0, ADD, MAX)

                def evict(co, h, pb_slices):
                    """pb_slices: list of (psum_ap, row0, nrow) to stage."""
                    ev = evpool.tile([H, HALF], F32,
                                     name=f"rep{rep}_ev_{co}_{h}", tag="ev")
                    for ap, row0, nrow in pb_slices:
                        dst = ev[row0:row0 + nrow, :]
                        if ev_mode == "act":
                            nc.scalar.copy(dst, ap)
                        elif ev_mode == "pool":
                            nc.gpsimd.tensor_copy(dst, ap)
                        else:
                            nc.vector.tensor_copy(dst, ap)
                    nc.sync.dma_start(
                        logits_d[co * COHORT:(co + 1) * COHORT,
                                 h * HALF:(h + 1) * HALF], ev[:, :])

                if dense:
                    with tc.tile_pool(name=f"rep{rep}_plog", bufs=4,
                                      space="PSUM") as plog:
                        for co in range(n_cohorts):
                            pbank = [plog.tile([H, HALF], F32,
                                               name=f"rep{rep}_pb_{co}_{h}",
                                               tag="plog") for h in range(2)]
                            # init waves: psum <- b2 (rank-1 b2 x ones)
                            for j in range(GROUPS):
                                for h in range(2):
                                    nc.tensor.matmul(
                                        pbank[h][32 * j:32 * j + 32, :],
                                        b2row_sb[0:1, 32 * j:32 * j + 32],
                                        ones1_sb[0:1, :],
                                        start=True, stop=False,
                                        tile_position=(0, 32 * j),
                                        skip_group_check=True)
                            for r in range(WAVES):
                                for j in range(GROUPS):
                                    ul = 32 * j + r
                                    u = co * COHORT + ul
                                    rt = rpool.tile([H, C], F16,
                                                    name=f"rep{rep}_R_{u}",
                                                    tag="R")
                                    produce(u, r * GROUPS + j, rt)
                                    lhsT = w2big_sb[:, 32 * r:32 * r + 32]
                                    for h in range(2):
                                        nc.tensor.matmul(
                                            pbank[h][32 * j:32 * j + 32, :],
                                            lhsT,
                                            rt[:, h * HALF:(h + 1) * HALF],
                                            start=False,
                                            stop=(r == WAVES - 1),
                                            tile_position=(0, 32 * j),
                                            skip_group_check=True)
                            for h in range(2):
                                evict(co, h, [(pbank[h][:, :], 0, H)])
                else:
                    with tc.tile_pool(name=f"rep{rep}_plog", bufs=8,
                                      space="PSUM") as plog:
                        for co in range(n_cohorts):
                            pbank = [[plog.tile(
                                [H, HALF], F32,
                                name=f"rep{rep}_pb_{co}_{j}_{h}", tag="plog")
                                for h in range(2)] for j in range(GROUPS)]
                            for j in range(GROUPS):
                                for h in range(2):
                                    nc.tensor.matmul(
                                        pbank[j][h][32 * j:32 * j + 32, :],
                                        b2row_sb[0:1, 32 * j:32 * j + 32],
                                        ones1_sb[0:1, :],
                                        start=True, stop=False,
                                        tile_position=(0, 32 * j),
                                        skip_group_check=True)
                            for r in range(WAVES):
                                for j in range(GROUPS):
                                    ul = 32 * j + r
                                    u = co * COHORT + ul
                                    rt = rpool.tile([H, C], F16,
                                                    name=f"rep{rep}_R_{u}",
                                                    tag="R")
                                    produce(u, r * GROUPS + j, rt)
                                    lhsT = w2big_sb[:, 32 * r:32 * r + 32]
                                    for h in range(2):
                                        nc.tensor.matmul(
                                            pbank[j][h][32 * j:32 * j + 32, :],
                                            lhsT,
                                            rt[:, h * HALF:(h + 1) * HALF],
                                            start=False,
                                            stop=(r == WAVES - 1),
                                            tile_position=(0, 32 * j),
                                            skip_group_check=True)
                            for h in range(2):
                                evict(co, h,
                                      [(pbank[j][h][32 * j:32 * j + 32, :],
                                        32 * j, 32) for j in range(GROUPS)])

    nc.compile()
    _CACHE[key] = nc
    return nc


def prep_inputs(h_u_bar, item_content, cold_ids, Wi, bi, W1, b1, W2, b2):
    """Host-side shard/replicate prep. Returns per-core in_maps."""
    f32 = np.float32
    h_u_bar = np.asarray(h_u_bar, f32)
    item_content = np.asarray(item_content, f32)
    cold_ids = np.asarray(cold_ids)
    Wi = np.asarray(Wi, f32)
    bi = np.asarray(bi, f32)
    W1 = np.asarray(W1, f32)
    b1 = np.asarray(b1, f32)
    W2 = np.asarray(W2, f32)
    b2 = np.asarray(b2, f32)

    xcg = item_content[cold_ids]                       # (C, CD) gather
    xcgT = np.ascontiguousarray(xcg.T).astype(np.float16)
    wiT = np.ascontiguousarray(Wi.T).astype(np.float16)
    w1hT = np.ascontiguousarray(W1[:, :D].T)           # (D, H)
    w1xT = np.ascontiguousarray(W1[:, D:].T)           # (D, H)
    b1col = np.ascontiguousarray(b1[:, None])
    bicol = np.ascontiguousarray(bi[:, None])

    w2big = np.zeros((H, WAVES * 32), np.float16)
    for r in range(WAVES):
        w2big[:, 32 * r + r] = W2[0].astype(np.float16)
    b2row = np.full((1, 128), b2[0], np.float16)
    ones1 = np.ones((1, HALF), np.float16)

    common = dict(xcgT=xcgT, wiT=wiT, bicol=bicol, w1xT=w1xT, w1hT=w1hT,
                  b1col=b1col, w2big=w2big, b2row=b2row, ones1=ones1)
    in_maps = []
    for c in range(N_CORES):
        huT = np.ascontiguousarray(h_u_bar[c * UL:(c + 1) * UL].T)  # (D, UL)
        in_maps.append(dict(common, huT=huT))
    return in_maps


LAST_RESULTS = None  # BassKernelResults of the most recent run (for test.py)


def kernel(h_u_bar, item_content, cold_ids, Wi, bi, W1, b1, W2, b2,
           trace=False, trace_kwargs=None):
    global LAST_RESULTS
    from concourse.bass_utils import run_bass_kernel_spmd

    nc = build_bass()
    in_maps = prep_inputs(h_u_bar, item_content, cold_ids, Wi, bi, W1, b1,
                          W2, b2)
    kw = {}
    if trace:
        kw["trace"] = True
        if trace_kwargs:
            kw.update(trace_kwargs)
    res = run_bass_kernel_spmd(nc, in_maps, core_ids=list(range(N_CORES)), **kw)
    LAST_RESULTS = res
    out = np.concatenate([res.results[c]["logits"] for c in range(N_CORES)],
                         axis=0)
    return out.astype(np.float32)
